# revision 21
# baseline (speedup 1.0000x reference)
"""Trainium2 Bass kernel for nn_CrossAttentionLayer (B=8, N=2048, Q=256, D=1024, H=16).

Data-parallel over batch: 1 sample per NeuronCore, 8 cores, no collectives.

Math identities (host-side folds):
  - b_k dropped (constant shift along the softmax axis)
  - b_v folded through out_proj: bout_eff = b_out + W_o b_v (softmax rows sum to 1)
  - b_q and the 1/sqrt(HD) scale folded into the Q-projection eviction
  - exp computed as exp(s - 3): the e^-3 cancels in softmax normalization and
    keeps probs inside fp8-e4m3 normal range (max score ~5.5 -> e^2.5 ~ 12)

Precision plan (rel-err gate is 2e-2; residual dominates the output):
  - K/V/Q/out projections in fp8 e4m3 (weights host-scaled x32, unscaled at
    psum eviction); K and V use DoubleRow (2 k-tiles per matmul ~ 2x PE rate)
  - scores in bf16 with per-head-pair ROW TILING: head 2p on PE rows 0-63,
    head 2p+1 on rows 64-127, running concurrently (contraction = HD = 64)
  - probs in fp8 from ACT exp; attn@V in fp8 DoubleRow with a ones column
    appended to V (row 64 of the psum = softmax denominator, free)
  - normalization: batched reciprocal of denominators + PE broadcast (x32 to
    re-center fp8) + one DVE mul per pair

Engine budget per core: PE ~103us, ACT (exp only) ~73us, DVE (all psum
evictions) ~55us, DMA ~8.3MB in. ACT table + HAM warm-up during initial DMA.
"""

import numpy as np
import ml_dtypes
from contextlib import ExitStack

import concourse.bass as bass
import concourse.mybir as mybir
import concourse.tile as tile
from concourse import bacc
from concourse.bass_utils import run_bass_kernel_spmd

F32 = mybir.dt.float32
BF16 = mybir.dt.bfloat16
FP8 = mybir.dt.float8e4
AF = mybir.ActivationFunctionType
ALU = mybir.AluOpType
DR = mybir.MatmulPerfMode.DoubleRow

NP_FP8 = ml_dtypes.float8_e4m3
NP_BF16 = ml_dtypes.bfloat16

B, N, Q, D, H = 8, 2048, 256, 1024, 16
HD = D // H            # 64
KT = D // 128          # 8 contraction tiles
MT = D // 128          # 8 output tiles
NT = N // 128          # 16 token tiles
NW = N // 512          # 4 token windows (DMA + Kproj chunking)
PAIRS = H // 2         # 8 head pairs
WS = 32.0              # host weight pre-scale for fp8
N_CORES = 8


def build():
    nc = bacc.Bacc(None, target_bir_lowering=False)
    src8 = nc.declare_dram_parameter("src8", [NW, 128, KT, 512], FP8, isOutput=False)
    qry8 = nc.declare_dram_parameter("qry8", [128, KT, Q], FP8, isOutput=False)
    wk8 = nc.declare_dram_parameter("wk8", [128, KT, D], FP8, isOutput=False)
    wv8 = nc.declare_dram_parameter("wv8", [128, KT, D], FP8, isOutput=False)
    wq8 = nc.declare_dram_parameter("wq8", [128, KT, D], FP8, isOutput=False)
    wo8 = nc.declare_dram_parameter("wo8", [128, KT, D], FP8, isOutput=False)
    bq8 = nc.declare_dram_parameter("bq8", [128, MT], F32, isOutput=False)
    resid = nc.declare_dram_parameter("resid", [128, Q // 128, D], F32, isOutput=False)
    out = nc.declare_dram_parameter("out", [Q, D], F32, isOutput=True)

    with tile.TileContext(nc) as tc, ExitStack() as ctx:
        proj_ps = ctx.enter_context(tc.tile_pool(name="projps", bufs=2, space="PSUM"))
        sc_ps = ctx.enter_context(tc.tile_pool(name="scps", bufs=2, space="PSUM"))
        pso_ps = ctx.enter_context(tc.tile_pool(name="psops", bufs=2, space="PSUM"))

        big = ctx.enter_context(tc.tile_pool(name="big", bufs=1))
        exp_pool = ctx.enter_context(tc.tile_pool(name="expp", bufs=8))
        rb_pool = ctx.enter_context(tc.tile_pool(name="rbp", bufs=2))
        out_pool = ctx.enter_context(tc.tile_pool(name="outp", bufs=2))

        src_sb = big.tile([128, KT, NW, 512], FP8, tag="src")
        wk_sb = big.tile([128, KT, D], FP8, tag="wk")
        wv_sb = big.tile([128, KT, D], FP8, tag="wv")
        wq_sb = big.tile([128, KT, D], FP8, tag="wq")
        wo_sb = big.tile([128, KT, D], FP8, tag="wo")
        qry_sb = big.tile([128, KT, Q], FP8, tag="qry")
        bq_sb = big.tile([128, MT], F32, tag="bq")
        kt_sb = big.tile([128, MT, N], BF16, tag="kt")
        qt_sb = big.tile([128, MT, Q], BF16, tag="qt")
        v_sb = big.tile([128, NT, H, 66], FP8, tag="v")
        den_sb = big.tile([1, H, Q], BF16, tag="den")
        e32_sb = big.tile([1, 64], BF16, tag="e32")
        ao_sb = big.tile([128, MT, Q], BF16, tag="ao")
        ao8_sb = big.tile([128, MT, Q], FP8, tag="ao8")
        resid_sb = big.tile([128, Q // 128, D], F32, tag="res")
        warm_sb = big.tile([16, 512], BF16, tag="warm")
        warmo_sb = big.tile([16, 16], F32, tag="warmo")
        nb_sb = big.tile([128, 1], F32, tag="negbias")

        # ---- init: memsets (gpsimd), ACT exp-table warm, PE HAM warm ----
        nc.gpsimd.memset(warm_sb, 0.0)
        nc.vector.memset(warmo_sb, 0.0)
        nc.vector.memset(e32_sb, 1.0 / WS)  # denb = den/32 -> rb = 32/den
        nc.gpsimd.memset(v_sb[:, :, :, 64:65], 1.0)   # denominator ones column
        nc.gpsimd.memset(v_sb[:, :, :, 65:66], 0.0)   # padding
        nc.gpsimd.memset(nb_sb, -3.0)                 # exp re-centering bias
        # load the exp table set early (hides the ~2.7us ACT_TABLE_LOAD)
        nc.scalar.activation(out=warmo_sb[0:1, :], in_=warm_sb[0:1, 0:16], func=AF.Exp)
        # dummy matmuls to trip the PE HAM un-throttle during the DMA window
        for i in range(12):
            wp = proj_ps.tile([16, 512], F32, tag="proj", name=f"warm{i}")
            nc.tensor.matmul(wp[:], lhsT=warm_sb[:, 0:16], rhs=warm_sb[:], start=True, stop=True)

        # ---- DMA loads (sync queue), priority order ----
        nc.sync.dma_start(out=qry_sb, in_=qry8[:])
        nc.sync.dma_start(out=wq_sb[:, :, 0:512], in_=wq8[:, :, 0:512])
        nc.sync.dma_start(out=bq_sb, in_=bq8[:])
        nc.sync.dma_start(out=src_sb[:, :, 0, :], in_=src8[0])
        nc.sync.dma_start(out=wq_sb[:, :, 512:1024], in_=wq8[:, :, 512:1024])
        nc.sync.dma_start(out=wk_sb[:, :, 0:256], in_=wk8[:, :, 0:256])
        nc.sync.dma_start(out=src_sb[:, :, 1, :], in_=src8[1])
        nc.sync.dma_start(out=wk_sb[:, :, 256:1024], in_=wk8[:, :, 256:1024])
        nc.sync.dma_start(out=src_sb[:, :, 2, :], in_=src8[2])
        nc.sync.dma_start(out=src_sb[:, :, 3, :], in_=src8[3])
        nc.sync.dma_start(out=wv_sb, in_=wv8[:])
        nc.sync.dma_start(out=wo_sb, in_=wo8[:])
        nc.sync.dma_start(out=resid_sb, in_=resid[:])

        # ---- Q projection (plain fp8): qt = (psum/(WS*8)) + b_q/8 ----
        for m in range(MT):
            qp = proj_ps.tile([128, Q], F32, tag="proj", name=f"qp{m}")
            for k in range(KT):
                nc.tensor.matmul(
                    qp[:], lhsT=wq_sb[:, k, m * 128:(m + 1) * 128],
                    rhs=qry_sb[:, k, :], start=(k == 0), stop=(k == KT - 1),
                )
            nc.vector.tensor_scalar(
                out=qt_sb[:, m, :], in0=qp[:],
                scalar1=1.0 / (WS * 8.0), scalar2=bq_sb[:, m:m + 1],
                op0=ALU.mult, op1=ALU.add,
            )

        # ---- K projection (fp8 DoubleRow): kT[dout, tok] = W_k @ src^T ----
        def emit_kproj(m, w):
            kp = proj_ps.tile([128, 512], F32, tag="proj", name=f"kp{m}_{w}")
            for k in range(KT // 2):
                nc.tensor.matmul(
                    kp[:],
                    lhsT=wk_sb[:, 2 * k:2 * k + 2, m * 128:(m + 1) * 128],
                    rhs=src_sb[:, 2 * k:2 * k + 2, w, :],
                    start=(k == 0), stop=(k == KT // 2 - 1), perf_mode=DR,
                )
            nc.vector.tensor_scalar_mul(
                out=kt_sb[:, m, w * 512:(w + 1) * 512], in0=kp[:], scalar1=1.0 / WS
            )

        # ---- V projection (fp8 DoubleRow): v[tok, h, hd] = src @ W_v^T ----
        def emit_vproj(c, t):
            vp = proj_ps.tile([128, 512], F32, tag="proj", name=f"vp{c}_{t}")
            for k in range(KT // 2):
                nc.tensor.matmul(
                    vp[:],
                    lhsT=src_sb[:, 2 * k:2 * k + 2, t // 4, (t % 4) * 128:(t % 4) * 128 + 128],
                    rhs=wv_sb[:, 2 * k:2 * k + 2, c * 512:(c + 1) * 512],
                    start=(k == 0), stop=(k == KT // 2 - 1), perf_mode=DR,
                )
            nc.vector.tensor_scalar_mul(
                out=v_sb[:, t, c * 8:(c + 1) * 8, 0:64],
                in0=vp[:].rearrange("p (h d) -> p h d", h=8),
                scalar1=1.0 / WS,
            )

        # ---- attention: row-tiled scores + DR attn@V, emission-interleaved
        # with the K/V projections so ACT exp runs continuously ----
        expt = {}

        def emit_score_chunk(p, par, c):
            # 4 score matmuls + one exp for head 2p+par, n-tiles 4c..4c+3
            if (p, par) not in expt:
                expt[(p, par)] = exp_pool.tile(
                    [128, NT, Q], FP8, tag="exp", name=f"expt{p}_{par}"
                )
            po = par * 64
            sc = sc_ps.tile([128, 4, Q], F32, tag="sc", name=f"sc{p}_{c}_{par}")
            for j in range(4):
                nt = 4 * c + j
                nc.tensor.matmul(
                    sc[:, j, :],
                    lhsT=kt_sb[po:po + 64, p, nt * 128:(nt + 1) * 128],
                    rhs=qt_sb[po:po + 64, p, :],
                    start=True, stop=True,
                )
            nc.scalar.activation(
                out=expt[(p, par)][:, 4 * c:4 * c + 4, :], in_=sc[:],
                func=AF.Exp, bias=nb_sb[:],
            )

        def emit_scores_half(p, half):
            # half 0: chunks (e,0) (o,0) (e,1) (o,1); half 1: c = 2,3
            for c in (2 * half, 2 * half + 1):
                for par in range(2):
                    emit_score_chunk(p, par, c)

        def emit_attnv(p):
            for par in range(2):
                h = 2 * p + par
                pso = pso_ps.tile([65, Q], F32, tag="pso", name=f"pso{h}")
                for tt in range(NT // 2):
                    nc.tensor.matmul(
                        pso[:],
                        lhsT=v_sb[:, 2 * tt:2 * tt + 2, h, 0:65],
                        rhs=expt[(p, par)][:, 2 * tt:2 * tt + 2, :],
                        start=(tt == 0), stop=(tt == NT // 2 - 1), perf_mode=DR,
                    )
                nc.vector.tensor_copy(ao_sb[par * 64:par * 64 + 64, p, :], pso[0:64, :])
                nc.vector.tensor_copy(den_sb[:, h, :], pso[64:65, :])

        def emit_norm(p):
            denb = sc_ps.tile([128, Q], F32, tag="sc", name=f"denb{p}")
            for par in range(2):
                nc.tensor.matmul(
                    denb[par * 64:par * 64 + 64, :], lhsT=e32_sb[:],
                    rhs=den_sb[:, 2 * p + par, :], start=True, stop=True,
                )
            rb = rb_pool.tile([128, Q], F32, tag="rb", name=f"rb{p}")
            nc.vector.reciprocal_approx_fast(out=rb[:], in_=denb[:])
            nc.vector.tensor_mul(ao8_sb[:, p, :], ao_sb[:, p, :], rb[:])

        # fine-grained interleave: score half-blocks (PE-light, feeds ACT)
        # alternate with projection / attn@V / normalize blocks (PE-heavy),
        # so the in-order PE queue never waits on ACT psum recycling.
        def K(m):
            for w in range(NW):
                emit_kproj(m, w)

        def V(c, lo, hi):
            for t in range(lo, hi):
                emit_vproj(c, t)

        K(0); K(1)
        emit_scores_half(0, 0); K(2)
        emit_scores_half(0, 1); K(3)
        emit_scores_half(1, 0); K(4)
        emit_scores_half(1, 1); K(5)
        emit_scores_half(2, 0); V(0, 0, 8)
        emit_scores_half(2, 1); V(0, 8, 16)
        emit_scores_half(3, 0); K(6)
        emit_scores_half(3, 1); emit_attnv(0)
        emit_scores_half(4, 0); K(7)
        emit_scores_half(4, 1); emit_attnv(1)
        emit_scores_half(5, 0); V(1, 0, 8)
        emit_scores_half(5, 1); emit_attnv(2); emit_norm(0)
        emit_scores_half(6, 0); V(1, 8, 16)
        emit_scores_half(6, 1); emit_attnv(3); emit_norm(1)
        emit_scores_half(7, 0); emit_attnv(4); emit_norm(2)
        emit_scores_half(7, 1); emit_attnv(5); emit_norm(3)
        emit_attnv(6); emit_norm(4); emit_norm(5)
        emit_attnv(7); emit_norm(6); emit_norm(7)

        # ---- out projection (fp8 DoubleRow) + residual ----
        for qt in range(Q // 128):
            for c in range(2):
                op = proj_ps.tile([128, 512], F32, tag="proj", name=f"op{qt}_{c}")
                for m in range(MT // 2):
                    nc.tensor.matmul(
                        op[:],
                        lhsT=ao8_sb[:, 2 * m:2 * m + 2, qt * 128:(qt + 1) * 128],
                        rhs=wo_sb[:, 2 * m:2 * m + 2, c * 512:(c + 1) * 512],
                        start=(m == 0), stop=(m == MT // 2 - 1), perf_mode=DR,
                    )
                ot = out_pool.tile([128, 512], BF16, tag="ot", name=f"ot{qt}_{c}")
                nc.scalar.activation(out=ot[:], in_=op[:], func=AF.Copy, scale=1.0 / (WS * WS))
                of = out_pool.tile([128, 512], F32, tag="of", name=f"of{qt}_{c}")
                nc.vector.tensor_add(of[:], ot[:], resid_sb[:, qt, c * 512:(c + 1) * 512])
                nc.sync.dma_start(
                    out=out[qt * 128:(qt + 1) * 128, c * 512:(c + 1) * 512], in_=of
                )

    nc.finalize()
    return nc


_NC_CACHE = {}


def _get_nc():
    if "nc" not in _NC_CACHE:
        _NC_CACHE["nc"] = build()
    return _NC_CACHE["nc"]


def _fp8(x):
    return np.clip(x, -240.0, 240.0).astype(NP_FP8)


def make_in_maps(sources, queries, w_in, b_in, w_out, b_out):
    sources = np.asarray(sources, dtype=np.float32)
    queries = np.asarray(queries, dtype=np.float32)
    w_in = np.asarray(w_in, dtype=np.float32)
    b_in = np.asarray(b_in, dtype=np.float32)
    w_out = np.asarray(w_out, dtype=np.float32)
    b_out = np.asarray(b_out, dtype=np.float32)

    w_q, w_k, w_v = w_in[0:D], w_in[D:2 * D], w_in[2 * D:3 * D]
    b_q, b_v = b_in[0:D], b_in[2 * D:3 * D]
    bout_eff = b_out + w_out @ b_v

    def wprep(w):  # [dout, din] -> fp8 [128, KT, D] p-major of (w.T * WS)
        wt = np.ascontiguousarray(w.T) * WS
        return _fp8(wt.reshape(KT, 128, D).transpose(1, 0, 2))

    wk8 = wprep(w_k)
    wv8 = wprep(w_v)
    wq8 = wprep(w_q)
    wo8 = wprep(w_out)
    bq8 = (b_q / 8.0).reshape(MT, 128).transpose(1, 0).copy()

    in_maps = []
    for b in range(B):
        st = sources[b].T  # [D, N]
        src8 = _fp8(st.reshape(KT, 128, NW, 512).transpose(2, 1, 0, 3))
        qt = queries[b].T  # [D, Q]
        qry8 = _fp8(qt.reshape(KT, 128, Q).transpose(1, 0, 2))
        res = (queries[b] + bout_eff[None, :]).reshape(Q // 128, 128, D).transpose(1, 0, 2).copy()
        in_maps.append({
            "src8": src8, "qry8": qry8,
            "wk8": wk8, "wv8": wv8, "wq8": wq8, "wo8": wo8,
            "bq8": bq8, "resid": res,
        })
    return in_maps


def kernel(sources, queries, w_in, b_in, w_out, b_out, _trace=False):
    nc = _get_nc()
    in_maps = make_in_maps(sources, queries, w_in, b_in, w_out, b_out)
    res = run_bass_kernel_spmd(nc, in_maps, core_ids=list(range(N_CORES)), trace=_trace)
    out = np.stack([res.results[b]["out"] for b in range(B)], axis=0)
    if _trace:
        kernel.last_exec_time_ns = res.exec_time_ns
        kernel.last_results = res
    return out


# revision 22
# speedup vs baseline: 1.0114x; 1.0114x over previous
"""Trainium2 Bass kernel for nn_CrossAttentionLayer (B=8, N=2048, Q=256, D=1024, H=16).

Data-parallel over batch: 1 sample per NeuronCore, 8 cores, no collectives.

Math identities (host-side folds):
  - b_k dropped (constant shift along the softmax axis)
  - b_v folded through out_proj: bout_eff = b_out + W_o b_v (softmax rows sum to 1)
  - b_q and the 1/sqrt(HD) scale folded into the Q-projection eviction
  - exp computed as exp(s - 3): the e^-3 cancels in softmax normalization and
    keeps probs inside fp8-e4m3 normal range (max score ~5.5 -> e^2.5 ~ 12)

Precision plan (rel-err gate is 2e-2; residual dominates the output):
  - K/V/Q/out projections in fp8 e4m3 (weights host-scaled x32, unscaled at
    psum eviction); K and V use DoubleRow (2 k-tiles per matmul ~ 2x PE rate)
  - scores in bf16 with per-head-pair ROW TILING: head 2p on PE rows 0-63,
    head 2p+1 on rows 64-127, running concurrently (contraction = HD = 64)
  - probs in fp8 from ACT exp; attn@V in fp8 DoubleRow with a ones column
    appended to V (row 64 of the psum = softmax denominator, free)
  - normalization: batched reciprocal of denominators + PE broadcast (x32 to
    re-center fp8) + one DVE mul per pair

Engine budget per core: PE ~103us, ACT (exp only) ~73us, DVE (all psum
evictions) ~55us, DMA ~8.3MB in. ACT table + HAM warm-up during initial DMA.
"""

import numpy as np
import ml_dtypes
from contextlib import ExitStack

import concourse.bass as bass
import concourse.mybir as mybir
import concourse.tile as tile
from concourse import bacc
from concourse.bass_utils import run_bass_kernel_spmd

F32 = mybir.dt.float32
BF16 = mybir.dt.bfloat16
FP8 = mybir.dt.float8e4
AF = mybir.ActivationFunctionType
ALU = mybir.AluOpType
DR = mybir.MatmulPerfMode.DoubleRow

NP_FP8 = ml_dtypes.float8_e4m3
NP_BF16 = ml_dtypes.bfloat16

B, N, Q, D, H = 8, 2048, 256, 1024, 16
HD = D // H            # 64
KT = D // 128          # 8 contraction tiles
MT = D // 128          # 8 output tiles
NT = N // 128          # 16 token tiles
NW = N // 512          # 4 token windows (DMA + Kproj chunking)
PAIRS = H // 2         # 8 head pairs
WS = 32.0              # host weight pre-scale for fp8
N_CORES = 8


def build():
    nc = bacc.Bacc(None, target_bir_lowering=False)
    src8 = nc.declare_dram_parameter("src8", [NW, 128, KT, 512], FP8, isOutput=False)
    qry8 = nc.declare_dram_parameter("qry8", [128, KT, Q], FP8, isOutput=False)
    wk8 = nc.declare_dram_parameter("wk8", [128, KT, D], FP8, isOutput=False)
    wv8 = nc.declare_dram_parameter("wv8", [128, KT, D], FP8, isOutput=False)
    wq8 = nc.declare_dram_parameter("wq8", [128, KT, D], FP8, isOutput=False)
    wo8 = nc.declare_dram_parameter("wo8", [128, KT, D], FP8, isOutput=False)
    bq8 = nc.declare_dram_parameter("bq8", [128, MT], F32, isOutput=False)
    resid = nc.declare_dram_parameter("resid", [128, Q // 128, D], F32, isOutput=False)
    out = nc.declare_dram_parameter("out", [Q, D], F32, isOutput=True)

    with tile.TileContext(nc) as tc, ExitStack() as ctx:
        proj_ps = ctx.enter_context(tc.tile_pool(name="projps", bufs=2, space="PSUM"))
        sc_ps = ctx.enter_context(tc.tile_pool(name="scps", bufs=2, space="PSUM"))
        pso_ps = ctx.enter_context(tc.tile_pool(name="psops", bufs=2, space="PSUM"))

        big = ctx.enter_context(tc.tile_pool(name="big", bufs=1))
        exp_pool = ctx.enter_context(tc.tile_pool(name="expp", bufs=8))
        rb_pool = ctx.enter_context(tc.tile_pool(name="rbp", bufs=2))
        out_pool = ctx.enter_context(tc.tile_pool(name="outp", bufs=2))

        src_sb = big.tile([128, KT, NW, 512], FP8, tag="src")
        wk_sb = big.tile([128, KT, D], FP8, tag="wk")
        wv_sb = big.tile([128, KT, D], FP8, tag="wv")
        wq_sb = big.tile([128, KT, D], FP8, tag="wq")
        wo_sb = big.tile([128, KT, D], FP8, tag="wo")
        qry_sb = big.tile([128, KT, Q], FP8, tag="qry")
        bq_sb = big.tile([128, MT], F32, tag="bq")
        kt_sb = big.tile([128, MT, N], BF16, tag="kt")
        qt_sb = big.tile([128, MT, Q], BF16, tag="qt")
        v_sb = big.tile([128, NT, H, 66], FP8, tag="v")
        den_sb = big.tile([1, H, Q], BF16, tag="den")
        e32_sb = big.tile([1, 64], BF16, tag="e32")
        ao_sb = big.tile([128, MT, Q], BF16, tag="ao")
        ao8_sb = big.tile([128, MT, Q], FP8, tag="ao8")
        resid_sb = big.tile([128, Q // 128, D], F32, tag="res")
        warm_sb = big.tile([16, 512], BF16, tag="warm")
        warmo_sb = big.tile([16, 16], F32, tag="warmo")
        nb_sb = big.tile([128, 1], F32, tag="negbias")

        # ---- init: memsets (gpsimd), ACT exp-table warm, PE HAM warm ----
        nc.gpsimd.memset(warm_sb, 0.0)
        nc.vector.memset(warmo_sb, 0.0)
        nc.vector.memset(e32_sb, 1.0 / WS)  # denb = den/32 -> rb = 32/den
        nc.gpsimd.memset(v_sb[:, :, :, 64:65], 1.0)   # denominator ones column
        nc.gpsimd.memset(v_sb[:, :, :, 65:66], 0.0)   # padding
        nc.gpsimd.memset(nb_sb, -3.0)                 # exp re-centering bias
        # load the exp table set early (hides the ~2.7us ACT_TABLE_LOAD)
        nc.scalar.activation(out=warmo_sb[0:1, :], in_=warm_sb[0:1, 0:16], func=AF.Exp)
        # dummy matmuls to trip the PE HAM un-throttle during the DMA window
        for i in range(16):
            wp = proj_ps.tile([16, 512], F32, tag="proj", name=f"warm{i}")
            nc.tensor.matmul(wp[:], lhsT=warm_sb[:, 0:16], rhs=warm_sb[:], start=True, stop=True)

        # ---- DMA loads (sync queue), priority order ----
        nc.sync.dma_start(out=qry_sb, in_=qry8[:])
        nc.sync.dma_start(out=wq_sb, in_=wq8[:])
        nc.sync.dma_start(out=bq_sb, in_=bq8[:])
        for w in range(NW):
            nc.sync.dma_start(out=src_sb[:, :, w, :], in_=src8[w])
        nc.sync.dma_start(out=wk_sb, in_=wk8[:])
        nc.sync.dma_start(out=wv_sb, in_=wv8[:])
        nc.sync.dma_start(out=wo_sb, in_=wo8[:])
        nc.sync.dma_start(out=resid_sb, in_=resid[:])

        # ---- Q projection (plain fp8): qt = (psum/(WS*8)) + b_q/8 ----
        for m in range(MT):
            qp = proj_ps.tile([128, Q], F32, tag="proj", name=f"qp{m}")
            for k in range(KT):
                nc.tensor.matmul(
                    qp[:], lhsT=wq_sb[:, k, m * 128:(m + 1) * 128],
                    rhs=qry_sb[:, k, :], start=(k == 0), stop=(k == KT - 1),
                )
            nc.vector.tensor_scalar(
                out=qt_sb[:, m, :], in0=qp[:],
                scalar1=1.0 / (WS * 8.0), scalar2=bq_sb[:, m:m + 1],
                op0=ALU.mult, op1=ALU.add,
            )

        # ---- K projection (fp8 DoubleRow): kT[dout, tok] = W_k @ src^T ----
        def emit_kproj(m, w):
            kp = proj_ps.tile([128, 512], F32, tag="proj", name=f"kp{m}_{w}")
            for k in range(KT // 2):
                nc.tensor.matmul(
                    kp[:],
                    lhsT=wk_sb[:, 2 * k:2 * k + 2, m * 128:(m + 1) * 128],
                    rhs=src_sb[:, 2 * k:2 * k + 2, w, :],
                    start=(k == 0), stop=(k == KT // 2 - 1), perf_mode=DR,
                )
            nc.vector.tensor_scalar_mul(
                out=kt_sb[:, m, w * 512:(w + 1) * 512], in0=kp[:], scalar1=1.0 / WS
            )

        # ---- V projection (fp8 DoubleRow): v[tok, h, hd] = src @ W_v^T ----
        def emit_vproj(c, t):
            vp = proj_ps.tile([128, 512], F32, tag="proj", name=f"vp{c}_{t}")
            for k in range(KT // 2):
                nc.tensor.matmul(
                    vp[:],
                    lhsT=src_sb[:, 2 * k:2 * k + 2, t // 4, (t % 4) * 128:(t % 4) * 128 + 128],
                    rhs=wv_sb[:, 2 * k:2 * k + 2, c * 512:(c + 1) * 512],
                    start=(k == 0), stop=(k == KT // 2 - 1), perf_mode=DR,
                )
            nc.vector.tensor_scalar_mul(
                out=v_sb[:, t, c * 8:(c + 1) * 8, 0:64],
                in0=vp[:].rearrange("p (h d) -> p h d", h=8),
                scalar1=1.0 / WS,
            )

        # ---- attention: row-tiled scores + DR attn@V, emission-interleaved
        # with the K/V projections so ACT exp runs continuously ----
        expt = {}

        def emit_score_chunk(p, par, c):
            # 4 score matmuls + one exp for head 2p+par, n-tiles 4c..4c+3
            if (p, par) not in expt:
                expt[(p, par)] = exp_pool.tile(
                    [128, NT, Q], FP8, tag="exp", name=f"expt{p}_{par}"
                )
            po = par * 64
            sc = sc_ps.tile([128, 4, Q], F32, tag="sc", name=f"sc{p}_{c}_{par}")
            for j in range(4):
                nt = 4 * c + j
                nc.tensor.matmul(
                    sc[:, j, :],
                    lhsT=kt_sb[po:po + 64, p, nt * 128:(nt + 1) * 128],
                    rhs=qt_sb[po:po + 64, p, :],
                    start=True, stop=True,
                )
            nc.scalar.activation(
                out=expt[(p, par)][:, 4 * c:4 * c + 4, :], in_=sc[:],
                func=AF.Exp, bias=nb_sb[:],
            )

        def emit_scores_half(p, half):
            # half 0: chunks (e,0) (o,0) (e,1) (o,1); half 1: c = 2,3
            for c in (2 * half, 2 * half + 1):
                for par in range(2):
                    emit_score_chunk(p, par, c)

        def emit_attnv(p):
            for par in range(2):
                h = 2 * p + par
                pso = pso_ps.tile([65, Q], F32, tag="pso", name=f"pso{h}")
                for tt in range(NT // 2):
                    nc.tensor.matmul(
                        pso[:],
                        lhsT=v_sb[:, 2 * tt:2 * tt + 2, h, 0:65],
                        rhs=expt[(p, par)][:, 2 * tt:2 * tt + 2, :],
                        start=(tt == 0), stop=(tt == NT // 2 - 1), perf_mode=DR,
                    )
                nc.vector.tensor_copy(ao_sb[par * 64:par * 64 + 64, p, :], pso[0:64, :])
                nc.vector.tensor_copy(den_sb[:, h, :], pso[64:65, :])

        def emit_norm(p):
            denb = proj_ps.tile([128, Q], F32, tag="proj", name=f"denb{p}")
            for par in range(2):
                nc.tensor.matmul(
                    denb[par * 64:par * 64 + 64, :], lhsT=e32_sb[:],
                    rhs=den_sb[:, 2 * p + par, :], start=True, stop=True,
                )
            rb = rb_pool.tile([128, Q], F32, tag="rb", name=f"rb{p}")
            nc.vector.reciprocal_approx_fast(out=rb[:], in_=denb[:])
            nc.vector.tensor_mul(ao8_sb[:, p, :], ao_sb[:, p, :], rb[:])

        # fine-grained interleave: score half-blocks (PE-light, feeds ACT)
        # alternate with projection / attn@V / normalize blocks (PE-heavy),
        # so the in-order PE queue never waits on ACT psum recycling.
        def K(m):
            for w in range(NW):
                emit_kproj(m, w)

        def V(c, lo, hi):
            for t in range(lo, hi):
                emit_vproj(c, t)

        K(0); K(1)
        emit_scores_half(0, 0); K(2)
        emit_scores_half(0, 1); K(3)
        emit_scores_half(1, 0); K(4)
        emit_scores_half(1, 1); K(5)
        emit_scores_half(2, 0); V(0, 0, 8)
        emit_scores_half(2, 1); V(0, 8, 16)
        emit_scores_half(3, 0); K(6)
        emit_scores_half(3, 1); emit_attnv(0)
        emit_scores_half(4, 0); K(7)
        emit_scores_half(4, 1); emit_attnv(1)
        emit_scores_half(5, 0); V(1, 0, 8)
        emit_scores_half(5, 1); emit_attnv(2); emit_norm(0)
        emit_scores_half(6, 0); V(1, 8, 16)
        emit_scores_half(6, 1); emit_attnv(3); emit_norm(1)
        emit_scores_half(7, 0); emit_attnv(4); emit_norm(2)
        emit_scores_half(7, 1); emit_attnv(5); emit_norm(3)
        emit_attnv(6); emit_norm(4); emit_norm(5)
        emit_attnv(7); emit_norm(6); emit_norm(7)

        # ---- out projection (fp8 DoubleRow) + residual ----
        for qt in range(Q // 128):
            for c in range(2):
                op = proj_ps.tile([128, 512], F32, tag="proj", name=f"op{qt}_{c}")
                for m in range(MT // 2):
                    nc.tensor.matmul(
                        op[:],
                        lhsT=ao8_sb[:, 2 * m:2 * m + 2, qt * 128:(qt + 1) * 128],
                        rhs=wo_sb[:, 2 * m:2 * m + 2, c * 512:(c + 1) * 512],
                        start=(m == 0), stop=(m == MT // 2 - 1), perf_mode=DR,
                    )
                ot = out_pool.tile([128, 512], BF16, tag="ot", name=f"ot{qt}_{c}")
                nc.scalar.activation(out=ot[:], in_=op[:], func=AF.Copy, scale=1.0 / (WS * WS))
                of = out_pool.tile([128, 512], F32, tag="of", name=f"of{qt}_{c}")
                nc.vector.tensor_add(of[:], ot[:], resid_sb[:, qt, c * 512:(c + 1) * 512])
                nc.sync.dma_start(
                    out=out[qt * 128:(qt + 1) * 128, c * 512:(c + 1) * 512], in_=of
                )

    nc.finalize()
    return nc


_NC_CACHE = {}


def _get_nc():
    if "nc" not in _NC_CACHE:
        _NC_CACHE["nc"] = build()
    return _NC_CACHE["nc"]


def _fp8(x):
    return np.clip(x, -240.0, 240.0).astype(NP_FP8)


def make_in_maps(sources, queries, w_in, b_in, w_out, b_out):
    sources = np.asarray(sources, dtype=np.float32)
    queries = np.asarray(queries, dtype=np.float32)
    w_in = np.asarray(w_in, dtype=np.float32)
    b_in = np.asarray(b_in, dtype=np.float32)
    w_out = np.asarray(w_out, dtype=np.float32)
    b_out = np.asarray(b_out, dtype=np.float32)

    w_q, w_k, w_v = w_in[0:D], w_in[D:2 * D], w_in[2 * D:3 * D]
    b_q, b_v = b_in[0:D], b_in[2 * D:3 * D]
    bout_eff = b_out + w_out @ b_v

    def wprep(w):  # [dout, din] -> fp8 [128, KT, D] p-major of (w.T * WS)
        wt = np.ascontiguousarray(w.T) * WS
        return _fp8(wt.reshape(KT, 128, D).transpose(1, 0, 2))

    wk8 = wprep(w_k)
    wv8 = wprep(w_v)
    wq8 = wprep(w_q)
    wo8 = wprep(w_out)
    bq8 = (b_q / 8.0).reshape(MT, 128).transpose(1, 0).copy()

    in_maps = []
    for b in range(B):
        st = sources[b].T  # [D, N]
        src8 = _fp8(st.reshape(KT, 128, NW, 512).transpose(2, 1, 0, 3))
        qt = queries[b].T  # [D, Q]
        qry8 = _fp8(qt.reshape(KT, 128, Q).transpose(1, 0, 2))
        res = (queries[b] + bout_eff[None, :]).reshape(Q // 128, 128, D).transpose(1, 0, 2).copy()
        in_maps.append({
            "src8": src8, "qry8": qry8,
            "wk8": wk8, "wv8": wv8, "wq8": wq8, "wo8": wo8,
            "bq8": bq8, "resid": res,
        })
    return in_maps


def kernel(sources, queries, w_in, b_in, w_out, b_out, _trace=False):
    nc = _get_nc()
    in_maps = make_in_maps(sources, queries, w_in, b_in, w_out, b_out)
    res = run_bass_kernel_spmd(nc, in_maps, core_ids=list(range(N_CORES)), trace=_trace)
    out = np.stack([res.results[b]["out"] for b in range(B)], axis=0)
    if _trace:
        kernel.last_exec_time_ns = res.exec_time_ns
        kernel.last_results = res
    return out
